# revision 12
# baseline (speedup 1.0000x reference)
"""Bahdanau-attention scoring kernel for Trainium2 (8 NeuronCores, data-parallel over batch).

Computes, for enc [S=2048, B=64, F=1024] f32 and hidden [B, 512] f32:
    energy    = tanh(cat([hidden_bcast, enc]) @ attn_w.T + attn_b)   # [S, B, 512]
    attention = energy @ v_w.T (+ v_b)                                # [S, B, 1]
    out       = softmax_over_S(attention / TEMP)                      # [S, B, 1]

v_b is a global scalar shift -> cancels in the softmax, dropped.

Transposed-stationary design: the PE matmuls put TOKENS on the output
partitions (stationary lhsT = 128-token fp8 block of x', moving rhs = the
attn weights). The energy PSUM comes out [128 tokens, 512 d], so:
  - the v-dot is a free-axis fused multiply+row-sum on the Vector engine
    (scalar_tensor_tensor with accum_out against a partition-replicated v;
    NB tensor_tensor_reduce wedges TRN2 hw) -- ZERO extra PE matmuls;
  - exp/softmax bookkeeping runs on 128 partitions (one small ACT exp per
    batch row with fused row-sum) instead of a single partition.
The per-partition ACT bias can't add h_proj (it varies along the free axis
here), so the hidden projection is folded into the DATA on the host:
    x' = enc[:, b, :] + W_dev^+ @ (hidden[b] @ W_h.T + attn_b)
an exact least-norm solve against the fp8-QUANTIZED device weights (W_dev
is full row rank; residual ~1e-14). This inflates the fp8 quantization
noise of x' by ~1.3x, still well inside tolerance, and the v-dot becomes
exact fp16/f32.

DMA: the whole 16.75 MB per-core x' stream is SBUF-RESIDENT (128KB of the
208KB usable per partition) and loaded by a few BIG per-batch-row DMAs
(16KB contiguous per partition) issued up-front and ungated. Dribbled
per-tile 4KB-row DMAs gated on compute only reach ~112 GB/s under 8-core
load; big early issue measures 310 GB/s/core -> the input stream takes
~55us and hides entirely under the ~110us PE stream. (b0 is split per-st,
interleaved with the weights on a second queue, so the first matmul still
starts at ~9us.)

The softmax normalization (a per-b scalar divide) happens in the host's
gather/transpose pass: the device ships exp(att/TEMP) [128, b, 16] plus
per-partition row sums; the host divides by the total.

Weights are prescaled x32 against e4m3 subnormals (1/32 rides the tanh
activation scale).
"""
import os
import sys
import types

import numpy as np
import ml_dtypes

S = 2048
B = 64
F = 1024
D = 512
NCORES = 8
BLOC = B // NCORES  # 8
TEMP = 3.0
ST = 4          # s-tiles per batch row (S / 512)
TT = 512        # tokens per tile
Q = TT // 128   # 128-token blocks per tile
KP = F // 256   # 4 contraction chunks (256 features each, fp8 DoubleRow)
WSCALE = 32.0   # fp8 weight prescale (attn_w values are subnormal in e4m3 otherwise)


def _install_ntff_hook():
    """Make trace=True work under axon by registering the NTFF profile hook."""
    try:
        from antenv import axon_hooks  # noqa: F401
        return
    except ImportError:
        pass
    try:
        import antenv
        from trn_agent_boot.trn_boot import _ntff_profile_via_ctypes
        mod = types.ModuleType("antenv.axon_hooks")
        mod._hook = _ntff_profile_via_ctypes("/opt/axon/libaxon_pjrt.so")
        mod.set_axon_ntff_profile_hook = lambda h: setattr(mod, "_hook", h)
        mod.get_axon_ntff_profile_hook = lambda: mod._hook
        sys.modules["antenv.axon_hooks"] = mod
        antenv.axon_hooks = mod
    except Exception:
        pass


_NC_CACHE = {}


def _build():
    if "nc" in _NC_CACHE:
        return _NC_CACHE["nc"]
    import concourse.bacc as bacc
    import concourse.mybir as mybir
    from concourse.tile import TileContext

    f32 = mybir.dt.float32
    fp16 = mybir.dt.float16
    fp8 = mybir.dt.float8e4

    nc = bacc.Bacc("TRN2")
    # x' blocks: xin[b, p, st, kp, q, j, t]
    #   = x'[f = 256*kp + 2*p + j, s = st*512 + q*128 + t]  for batch row b.
    # Per-b slice is 16KB contiguous per partition -> big-packet DMA.
    xin = nc.dram_tensor("xin", [BLOC, 128, ST, KP, Q, 2, 128], fp8,
                         kind="ExternalInput")
    # moving weights: wt[p, kp, d, j] = fp8(32 * w_e[d, 256*kp + 2*p + j])
    wt = nc.dram_tensor("wt", [128, KP, D, 2], fp8, kind="ExternalInput")
    # v replicated across partitions
    vrep = nc.dram_tensor("vrep", [128, D], fp16, kind="ExternalInput")
    # output: per (token-part, b): 16 cols exp(att/TEMP) + col 16 = row-sum.
    # One consolidated tensor -> one big-row DMA (per-b 64B-row DMAs crawl
    # at ~300ns/packet and stall the end-of-kernel barrier by ~8us).
    outd = nc.dram_tensor("out", [128, BLOC, ST * Q + 1], f32, kind="ExternalOutput")

    tiles = [(b, st) for b in range(BLOC) for st in range(ST)]

    with TileContext(nc) as tc:
        with (
            tc.tile_pool(name="consts", bufs=1) as cpool,
            tc.tile_pool(name="work", bufs=1) as pool,
            tc.tile_pool(name="ps_e", bufs=6, space="PSUM") as pse,
        ):
            # whole x' stream resident in SBUF: 128KB/partition
            xt = cpool.tile([128, BLOC, ST, KP, Q, 2, 128], fp8)
            wt_sb = cpool.tile([128, KP, D, 2], fp8)
            vrep_sb = cpool.tile([128, D], fp16)

            # Up-front, ungated DMA issue. sync queue: the x' stream (b0/st0
            # split per-kp so compute starts early, the rest coarse).
            # scalar queue: weights (one 4KB-row DMA) + vrep.
            xc = xt.rearrange("p b st kp q j t -> p b st kp (q j t)")
            xv = xt.rearrange("p b st kp q j t -> p b st (kp q j t)")
            xin_c = xin[0].rearrange("p st kp q j t -> p st kp (q j t)")
            nc.scalar.dma_start(
                out=wt_sb.rearrange("p kp d j -> p (kp d j)"),
                in_=wt.rearrange("p kp d j -> p (kp d j)"),
            )
            for kp in range(KP):
                nc.sync.dma_start(out=xc[:, 0, 0, kp], in_=xin_c[:, 0, kp])
            for st in range(1, ST):
                nc.sync.dma_start(
                    out=xv[:, 0, st],
                    in_=xin[0].rearrange("p st kp q j t -> p st (kp q j t)")[:, st],
                )
            nc.scalar.dma_start(out=vrep_sb[:], in_=vrep[:])
            xvb = xt.rearrange("p b st kp q j t -> p b (st kp q j t)")
            for b in range(1, BLOC):
                nc.sync.dma_start(
                    out=xvb[:, b],
                    in_=xin[b].rearrange("p st kp q j t -> p (st kp q j t)"),
                )

            # HAM warmup: keep the PE busy during the initial DMA wait so the
            # clock gate opens before the real stream starts (saves ~1.5us of
            # half-rate matmuls).
            warm = pool.tile([128, 128], fp8, tag="warm", bufs=1, name="warm")
            nc.vector.memset(warm[:], 0.25)
            with tc.tile_pool(name="ps_w", bufs=1, space="PSUM") as psw:
                wps = psw.tile([128, 128], f32, tag="wps", name="wps")
                for _ in range(14):
                    nc.tensor.matmul(wps[:], lhsT=warm[:], rhs=warm[:],
                                     start=True, stop=True)

            out_sb = pool.tile([128, BLOC, ST * Q + 1], f32, tag="osb", bufs=1,
                               name="osb")
            atts = {}

            for idx in range(len(tiles)):
                b, st = tiles[idx]
                if st == 0:
                    atts[b] = pool.tile([128, ST * Q], f32, tag="atts", bufs=2,
                                        name=f"at{b}")
                for q in range(Q):
                    ps = pse.tile([128, TT], f32, tag="ps", name="ps")
                    for kp in range(KP):
                        nc.tensor.matmul(
                            ps[:],
                            lhsT=xt[:, b, st, kp, q],
                            rhs=wt_sb[:, kp].rearrange("p d j -> p j d"),
                            start=(kp == 0),
                            stop=(kp == KP - 1),
                            perf_mode=mybir.MatmulPerfMode.DoubleRow,
                        )
                    th = pool.tile([128, TT], fp16, tag="th", bufs=4, name="th")
                    nc.scalar.activation(
                        th[:], ps[:], mybir.ActivationFunctionType.Tanh,
                        scale=float(1.0 / WSCALE),
                    )
                    # fused v-dot: out = th * vrep (scratch), accum = row-sum
                    wscr = pool.tile([128, TT], fp16, tag="wscr", bufs=2, name="wscr")
                    c = st * Q + q
                    nc.vector.scalar_tensor_tensor(
                        out=wscr[:],
                        in0=th[:],
                        scalar=0.0,
                        in1=vrep_sb[:],
                        op0=mybir.AluOpType.bypass,
                        op1=mybir.AluOpType.mult,
                        accum_out=atts[b][:, c : c + 1],
                    )
                if st == ST - 1:
                    nc.scalar.activation(
                        out_sb[:, b, : ST * Q], atts[b][:],
                        mybir.ActivationFunctionType.Exp,
                        scale=float(1.0 / TEMP),
                        accum_out=out_sb[:, b, ST * Q : ST * Q + 1],
                    )
                    if b == BLOC // 2 - 1:
                        # first half ships mid-stream (hidden)
                        nc.sync.dma_start(
                            out=outd[:, : BLOC // 2].rearrange("p b c -> p (b c)"),
                            in_=out_sb[:, : BLOC // 2].rearrange("p b c -> p (b c)"),
                        )
            # tail half split across both queues/partition ranges in parallel
            nc.sync.dma_start(
                out=outd[:64, BLOC // 2 :].rearrange("p b c -> p (b c)"),
                in_=out_sb[:64, BLOC // 2 :].rearrange("p b c -> p (b c)"),
            )
            nc.scalar.dma_start(
                out=outd[64:, BLOC // 2 :].rearrange("p b c -> p (b c)"),
                in_=out_sb[64:, BLOC // 2 :].rearrange("p b c -> p (b c)"),
            )

    nc.compile()
    _NC_CACHE["nc"] = nc
    return nc


def _prep(hidden, encoder_outputs, attn_w, attn_b, v_w):
    """Host prep: fold h_proj into x via least-norm solve vs quantized weights,
    quantize to fp8, and transpose to the per-tile stationary layout."""
    fp8np = ml_dtypes.float8_e4m3

    w_e = attn_w[:, D:]                               # [D, F]
    wt8 = (w_e * WSCALE).astype(fp8np)                # device weights
    w_dev = wt8.astype(np.float64) / WSCALE

    h_proj = hidden.astype(np.float64) @ attn_w[:, :D].T.astype(np.float64) + attn_b
    pinv = np.linalg.pinv(w_dev)                      # [F, D]
    dx = (pinv @ h_proj.T).T.astype(np.float32)       # [B, F]

    xq = (encoder_outputs + dx[None, :, :]).astype(fp8np)   # [S, B, F]
    # [S,B,F] -> [st, q, t, B, kp, p, j] -> [B, p, st, kp, q, j, t]
    v = xq.reshape(ST, Q, 128, B, KP, 128, 2).transpose(3, 5, 0, 4, 1, 6, 2)
    xin = np.ascontiguousarray(v)                     # [B, 128, ST, KP, Q, 2, 128]

    # wt[p, kp, d, j] = wt8[d, 256*kp + 2*p + j]
    wtl = np.ascontiguousarray(wt8.reshape(D, KP, 128, 2).transpose(2, 1, 0, 3))

    vrep = np.ascontiguousarray(
        np.broadcast_to(v_w[0].astype(np.float16)[None, :], (128, D))
    )
    return xin, wtl, vrep


def kernel(hidden, encoder_outputs, attn_w, attn_b, v_w, v_b):
    _install_ntff_hook()
    from concourse.bass_utils import run_bass_kernel_spmd

    hidden = np.asarray(hidden, dtype=np.float32)
    encoder_outputs = np.asarray(encoder_outputs, dtype=np.float32)
    attn_w = np.asarray(attn_w, dtype=np.float32)
    attn_b = np.asarray(attn_b, dtype=np.float32)
    v_w = np.asarray(v_w, dtype=np.float32)

    nc = _build()
    xin_full, wtl, vrep = _prep(hidden, encoder_outputs, attn_w, attn_b, v_w)

    in_maps = []
    for c in range(NCORES):
        b0 = c * BLOC
        in_maps.append(
            {
                "xin": np.ascontiguousarray(xin_full[b0 : b0 + BLOC]),
                "wt": wtl,
                "vrep": vrep,
            }
        )

    trace = bool(int(os.environ.get("KERNEL_TRACE", "0")))
    res = run_bass_kernel_spmd(
        nc, in_maps, core_ids=list(range(NCORES)), trace=trace
    )
    kernel.last_result = res

    outs = []
    for c in range(NCORES):
        ob = res.results[c]["out"]         # [128, BLOC, 17]
        ex = ob[:, :, : ST * Q]            # [128, BLOC, 16]
        sums = ob[:, :, ST * Q].astype(np.float64).sum(axis=0)  # [BLOC]
        o = ex / sums[None, :, None].astype(np.float32)
        # o[t, b, st*4+q] -> [b, s = st*512 + q*128 + t]
        o = o.reshape(128, BLOC, ST, Q).transpose(1, 2, 3, 0).reshape(BLOC, S)
        outs.append(o)
    full = np.concatenate(outs, axis=0)    # [B, S]
    full = full.transpose(1, 0).reshape(S, B, 1)
    return np.ascontiguousarray(full, dtype=np.float32)


kernel.last_result = None
